# revision 6
# baseline (speedup 1.0000x reference)
"""Causal self-attention (single head) Trainium2 Bass kernel.

Problem: x[4,2048,2048] -> qkv = x@W+b; q,k,v = split(qkv); causal
softmax attention with denominator output.
Returns (y[4,2048,2048], denom[4,2048]) like the reference.

Sharding: 8 cores = batch(4) x query-split(2). Within a batch pair,
each core takes the zebra half of the 16 query chunks (128 rows each):
r=0 -> chunks {g: g%4 in {0,3}}, r=1 -> {g: g%4 in {1,2}}. Both halves
have identical causal key-tile schedules [1,1,2,2,3,3,4,4] (512-wide
key tiles), so a single SPMD program serves all cores with no padding.

All matmuls run in float32r (TF32-like, full bf16 throughput at free
dim >= 256, ~1e-4 relative error).
"""

import sys

for _p in ("/opt/trn_rl_repo", "/root/.axon_site/_ro/trn_rl_repo"):
    if _p not in sys.path:
        sys.path.append(_p)

import numpy as np

import concourse.bass as bass
import concourse.tile as tile
from concourse import mybir
from concourse.bass_utils import run_bass_kernel_spmd

F32 = mybir.dt.float32
F32R = mybir.dt.float32r

B, N, E, D = 4, 2048, 2048, 2048
NCORES = 8
NQ = N // 2              # query rows per core
NCHUNK = 16              # 128-row chunks per batch
LCHUNK = 8               # local chunks per core
KTILE = 512
SCALE = 1.0 / float(np.sqrt(D))
NEG = -1.0e30

# zebra chunk assignment (identical schedule on every core)
CHUNKS = {
    0: [g for g in range(NCHUNK) if g % 4 in (0, 3)],
    1: [g for g in range(NCHUNK) if g % 4 in (1, 2)],
}
KCOUNTS = [g // 4 + 1 for g in CHUNKS[0]]
assert KCOUNTS == [g // 4 + 1 for g in CHUNKS[1]] == [1, 1, 2, 2, 3, 3, 4, 4]


def _split_waits(nc, limit=1):
    """This walrus build allows only one sem wait per instruction; move
    excess waits onto preceding NoOps on the same engine."""
    for bb in nc.main_func.blocks:
        newl = []
        for ins in bb.instructions:
            si = getattr(ins, "sync_info", None)
            if si is not None and len(si.on_wait) > limit:
                waits = list(si.on_wait)
                for i, w in enumerate(waits[:-limit]):
                    nd = mybir.InstNoOp(name=f"{ins.name}-ws{i}", ins=[], outs=[])
                    nd.engine = ins.engine
                    nd.sync_info = mybir.SyncInfo(on_update=[], on_wait=[w])
                    newl.append(nd)
                si.on_wait = waits[-limit:]
            newl.append(ins)
        bb.instructions = newl
    return nc


def _phase_a(nc, P, QT_d, KT_d, V_d, xT, xqT, W, bqk_sb, bv_sb):
    EC = E // 128
    # A1: Q^T. xq resident [128, EC, NQ] (shares the xbig slot)
    xq_sb = P["xbig"].tile([128, EC, N], F32R, tag="xbig")
    nc.sync.dma_start(xq_sb[:, :, :NQ], xqT.ap().rearrange("(a p) r -> p a r", p=128))
    for cc in range(16):
        w_sb = P["wtile"].tile([128, EC, 128], F32R)
        nc.sync.dma_start(
            w_sb[:],
            W.ap()[:, cc * 128 : (cc + 1) * 128].rearrange("(a p) c -> p a c", p=128),
        )
        for rt in range(NQ // KTILE):
            ps = P["psA"].tile([128, KTILE], F32)
            for ec in range(EC):
                nc.tensor.matmul(
                    ps[:], w_sb[:, ec, :],
                    xq_sb[:, ec, rt * KTILE : (rt + 1) * KTILE],
                    start=(ec == 0), stop=(ec == EC - 1),
                )
            o = P["projo"].tile([128, KTILE], F32R)
            nc.vector.tensor_scalar_add(o[:], ps[:], bqk_sb[:, cc : cc + 1])
            nc.sync.dma_start(
                QT_d[cc * 128 : (cc + 1) * 128, rt * KTILE : (rt + 1) * KTILE], o[:]
            )

    # A2: K^T. x resident (reuses xbig slot)
    x_sb = P["xbig"].tile([128, EC, N], F32R, tag="xbig")
    nc.sync.dma_start(x_sb[:], xT.ap().rearrange("(a p) r -> p a r", p=128))
    for cc in range(16):
        w_sb = P["wtile"].tile([128, EC, 128], F32R)
        nc.sync.dma_start(
            w_sb[:],
            W.ap()[:, D + cc * 128 : D + (cc + 1) * 128].rearrange(
                "(a p) c -> p a c", p=128
            ),
        )
        for rt in range(N // KTILE):
            ps = P["psA"].tile([128, KTILE], F32)
            for ec in range(EC):
                nc.tensor.matmul(
                    ps[:], w_sb[:, ec, :],
                    x_sb[:, ec, rt * KTILE : (rt + 1) * KTILE],
                    start=(ec == 0), stop=(ec == EC - 1),
                )
            o = P["projo"].tile([128, KTILE], F32R)
            nc.vector.tensor_scalar_add(o[:], ps[:], bqk_sb[:, 16 + cc : 17 + cc])
            nc.sync.dma_start(
                KT_d[cc * 128 : (cc + 1) * 128, rt * KTILE : (rt + 1) * KTILE], o[:]
            )

    # A3: V natural [rows, dcol]; stream W V-cols per 512-tile
    for vt in range(D // KTILE):
        wv_sb = P["wv"].tile([128, EC, KTILE], F32R)
        nc.sync.dma_start(
            wv_sb[:],
            W.ap()[:, 2 * D + vt * KTILE : 2 * D + (vt + 1) * KTILE].rearrange(
                "(a p) c -> p a c", p=128
            ),
        )
        for rc in range(16):
            ps = P["psA"].tile([128, KTILE], F32)
            for ec in range(EC):
                nc.tensor.matmul(
                    ps[:], x_sb[:, ec, rc * 128 : (rc + 1) * 128],
                    wv_sb[:, ec, :],
                    start=(ec == 0), stop=(ec == EC - 1),
                )
            o = P["projo"].tile([128, KTILE], F32R)
            nc.vector.tensor_add(o[:], ps[:], bv_sb[:, vt * KTILE : (vt + 1) * KTILE])
            nc.sync.dma_start(
                V_d[rc * 128 : (rc + 1) * 128, vt * KTILE : (vt + 1) * KTILE], o[:]
            )


def _attn_pass(nc, P, i, kt, kcount, qt_sb, kt_sb, v_sb, mask_sb,
               m_run, d_run, y_acc, ident_sb):
    EC = E // 128
    diag = kt == kcount - 1
    # QK^T
    ps_s = P["psS"].tile([128, KTILE], F32)
    for ec in range(EC):
        nc.tensor.matmul(
            ps_s[:], qt_sb[:, ec, i * 128 : (i + 1) * 128], kt_sb[:, ec, :],
            start=(ec == 0), stop=(ec == EC - 1),
        )
    if diag:
        src = P["soft"].tile([128, KTILE], F32, tag="masked")
        nc.vector.tensor_add(src[:], ps_s[:], mask_sb[i][:])
    else:
        src = ps_s
    # running max of raw logits
    mx = P["stats"].tile([128, 1], F32, tag="mx")
    nc.vector.tensor_reduce(
        mx[:], src[:], axis=mybir.AxisListType.X, op=mybir.AluOpType.max
    )
    if kt == 0:
        nc.vector.tensor_copy(m_run[i][:], mx[:])
    else:
        nc.vector.tensor_max(m_run[i][:], m_run[i][:], mx[:])
    # exp (no max subtraction; logits bounded) + denom partial
    p_sb = P["soft"].tile([128, KTILE], F32R, tag="p")
    dpart = P["stats"].tile([128, 1], F32, tag="dpart")
    nc.scalar.activation(
        p_sb[:], src[:], mybir.ActivationFunctionType.Exp,
        scale=SCALE, accum_out=dpart[:],
    )
    if kt == 0:
        nc.vector.tensor_copy(d_run[i][:], dpart[:])
    else:
        nc.vector.tensor_add(d_run[i][:], d_run[i][:], dpart[:])
    # transpose P -> PT (4 x 128x128)
    pt_sb = P["soft"].tile([128, 4, 128], F32R, tag="pt")
    for c in range(4):
        ps_t = P["psT"].tile([128, 128], F32R)
        nc.tensor.transpose(ps_t[:], p_sb[:, c * 128 : (c + 1) * 128], ident_sb[:])
        nc.vector.tensor_copy(pt_sb[:, c, :], ps_t[:])
    # PV for this key tile, accumulate into y_acc
    for dt in range(4):
        ps_y = P["psY"].tile([128, KTILE], F32)
        for c in range(4):
            nc.tensor.matmul(
                ps_y[:], pt_sb[:, c, :], v_sb[:, c, dt * KTILE : (dt + 1) * KTILE],
                start=(c == 0), stop=(c == 3),
            )
        dsl = slice(dt * KTILE, (dt + 1) * KTILE)
        if kt == 0:
            nc.vector.tensor_copy(y_acc[i][:, dsl], ps_y[:])
        else:
            nc.vector.tensor_add(y_acc[i][:, dsl], y_acc[i][:, dsl], ps_y[:])


def _phase_b(nc, P, QT_d, KT_d, V_d, msk, ident_sb, y_out, den_out):
    EC = E // 128
    for p in range(4):  # chunk pairs; lc = 2p, 2p+1; kcount = p+1
        kcount = p + 1
        qt_sb = P["qt"].tile([128, EC, 256], F32R)
        nc.sync.dma_start(
            qt_sb[:],
            QT_d[:, p * 256 : (p + 1) * 256].rearrange("(a p_) r -> p_ a r", p_=128),
        )
        y_acc = [P["yacc"].tile([128, D], F32, tag=f"yacc{i}", name=f"yacc{p}_{i}") for i in range(2)]
        m_run = [P["stats"].tile([128, 1], F32, tag=f"mrun{i}", name=f"mrun{p}_{i}") for i in range(2)]
        d_run = [P["stats"].tile([128, 1], F32, tag=f"drun{i}", name=f"drun{p}_{i}") for i in range(2)]
        mask_sb = [P["mskp"].tile([128, KTILE], F32, tag=f"msk{i}", name=f"msk{p}_{i}") for i in range(2)]
        for i in range(2):
            nc.sync.dma_start(mask_sb[i][:], msk.ap()[2 * p + i])

        for kt in range(kcount):
            kt_sb = P["kt"].tile([128, EC, KTILE], F32R, tag="ktile")
            nc.sync.dma_start(
                kt_sb[:],
                KT_d[:, kt * KTILE : (kt + 1) * KTILE].rearrange(
                    "(a p_) r -> p_ a r", p_=128
                ),
            )
            v_sb = P["vt"].tile([128, 4, D], F32R, tag="vtile")
            nc.sync.dma_start(
                v_sb[:],
                V_d[kt * KTILE : (kt + 1) * KTILE, :].rearrange(
                    "(c p_) d -> p_ c d", p_=128
                ),
            )
            for i in range(2):
                _attn_pass(nc, P, i, kt, kcount, qt_sb, kt_sb, v_sb, mask_sb,
                           m_run, d_run, y_acc, ident_sb)

        # pair epilogue: normalize + outputs
        for i in range(2):
            lc = 2 * p + i
            rden = P["stats"].tile([128, 1], F32, tag="rden")
            nc.vector.reciprocal(rden[:], d_run[i][:])
            yo = P["yo"].tile([128, D], F32, tag="yo")
            nc.vector.tensor_scalar_mul(yo[:], y_acc[i][:], rden[:])
            nc.sync.dma_start(y_out.ap()[lc * 128 : (lc + 1) * 128, :], yo[:])
            # denom matching reference: d_run * exp(-s*m_run)
            em = P["stats"].tile([128, 1], F32, tag="em")
            nc.scalar.activation(
                em[:], m_run[i][:], mybir.ActivationFunctionType.Exp, scale=-SCALE
            )
            dfin = P["stats"].tile([128, 1], F32, tag="dfin")
            nc.vector.tensor_mul(dfin[:], d_run[i][:], em[:])
            nc.sync.dma_start(den_out.ap()[lc * 128 : (lc + 1) * 128, :], dfin[:])


def build_nc():
    nc = bass.Bass("TRN2", target_bir_lowering=False, debug=False)

    xT = nc.dram_tensor("xT", [E, N], F32R, kind="ExternalInput")
    xqT = nc.dram_tensor("xqT", [E, NQ], F32R, kind="ExternalInput")
    W = nc.dram_tensor("W", [E, 3 * D], F32R, kind="ExternalInput")
    bqk = nc.dram_tensor("bqk", [128, 32], F32, kind="ExternalInput")
    bv = nc.dram_tensor("bv", [128, D], F32, kind="ExternalInput")
    msk = nc.dram_tensor("msk", [LCHUNK, 128, KTILE], F32, kind="ExternalInput")
    ident = nc.dram_tensor("ident", [128, 128], F32R, kind="ExternalInput")

    y_out = nc.dram_tensor("y_out", [NQ, D], F32, kind="ExternalOutput")
    den_out = nc.dram_tensor("den_out", [NQ, 1], F32, kind="ExternalOutput")

    with tile.TileContext(nc) as tc:
        with (
            tc.tile_pool(name="dram", bufs=1, space="DRAM") as dram,
            tc.tile_pool(name="const", bufs=1) as const_p,
        ):
            QT_d = dram.tile([D, NQ], F32R)
            KT_d = dram.tile([D, N], F32R)
            V_d = dram.tile([N, D], F32R)

            ident_sb = const_p.tile([128, 128], F32R)
            nc.sync.dma_start(ident_sb[:], ident.ap())

            with (
                tc.tile_pool(name="xbig", bufs=1) as xbig_p,
                tc.tile_pool(name="wtile", bufs=3) as wtile_p,
                tc.tile_pool(name="wv", bufs=1) as wv_p,
                tc.tile_pool(name="projo", bufs=3) as projo_p,
                tc.tile_pool(name="constA", bufs=1) as constA_p,
                tc.tile_pool(name="psA", bufs=4, space="PSUM") as psA,
            ):
                P = {"xbig": xbig_p, "wtile": wtile_p, "wv": wv_p,
                     "projo": projo_p, "psA": psA}
                bqk_sb = constA_p.tile([128, 32], F32)
                nc.sync.dma_start(bqk_sb[:], bqk.ap())
                bv_sb = constA_p.tile([128, D], F32)
                nc.sync.dma_start(bv_sb[:], bv.ap())
                _phase_a(nc, P, QT_d, KT_d, V_d, xT, xqT, W, bqk_sb, bv_sb)

            with (
                tc.tile_pool(name="kt", bufs=2) as kt_p,
                tc.tile_pool(name="vt", bufs=2) as vt_p,
                tc.tile_pool(name="qt", bufs=1) as qt_p,
                tc.tile_pool(name="yacc", bufs=1) as yacc_p,
                tc.tile_pool(name="soft", bufs=3) as soft_p,
                tc.tile_pool(name="yo", bufs=1) as yo_p,
                tc.tile_pool(name="stats", bufs=16) as stats_p,
                tc.tile_pool(name="mskp", bufs=1) as msk_p,
                tc.tile_pool(name="psS", bufs=2, space="PSUM") as psS,
                tc.tile_pool(name="psT", bufs=2, space="PSUM") as psT,
                tc.tile_pool(name="psY", bufs=2, space="PSUM") as psY,
            ):
                P = {"kt": kt_p, "vt": vt_p, "qt": qt_p, "yacc": yacc_p,
                     "soft": soft_p, "yo": yo_p, "stats": stats_p,
                     "mskp": msk_p, "psS": psS, "psT": psT, "psY": psY}
                _phase_b(nc, P, QT_d, KT_d, V_d, msk, ident_sb, y_out, den_out)

    _split_waits(nc)
    return nc


_NC_CACHE = None


def _get_nc():
    global _NC_CACHE
    if _NC_CACHE is None:
        _NC_CACHE = build_nc()
    return _NC_CACHE


def make_in_maps(x, W, b):
    x = np.asarray(x, dtype=np.float32)
    W = np.asarray(W, dtype=np.float32)
    b = np.asarray(b, dtype=np.float32)
    bqk = np.ascontiguousarray(b[: 2 * D].reshape(32, 128).T)
    bv = np.broadcast_to(b[2 * D :], (128, D)).copy()
    ident = np.eye(128, dtype=np.float32)
    in_maps = []
    for core in range(NCORES):
        bb, r = divmod(core, 2)
        chunks = CHUNKS[r]
        qrows = np.concatenate([np.arange(g * 128, (g + 1) * 128) for g in chunks])
        xT = np.ascontiguousarray(x[bb].T)
        xqT = np.ascontiguousarray(x[bb][qrows].T)
        msk = np.empty((LCHUNK, 128, KTILE), dtype=np.float32)
        for lc, g in enumerate(chunks):
            kc = KCOUNTS[lc]
            gi = g * 128 + np.arange(128)[:, None]
            kj = (kc - 1) * KTILE + np.arange(KTILE)[None, :]
            msk[lc] = np.where(kj <= gi, 0.0, NEG)
        in_maps.append(
            {"xT": xT, "xqT": xqT, "W": W, "bqk": bqk, "bv": bv,
             "msk": msk, "ident": ident}
        )
    return in_maps


def unshard(results):
    y = np.empty((B, N, D), dtype=np.float32)
    denom = np.empty((B, N), dtype=np.float32)
    for core in range(NCORES):
        bb, r = divmod(core, 2)
        chunks = CHUNKS[r]
        qrows = np.concatenate([np.arange(g * 128, (g + 1) * 128) for g in chunks])
        y[bb][qrows] = results[core]["y_out"]
        denom[bb][qrows] = results[core]["den_out"][:, 0]
    return y, denom


def kernel(x, W, b):
    nc = _get_nc()
    in_maps = make_in_maps(x, W, b)
    res = run_bass_kernel_spmd(nc, in_maps, list(range(NCORES)))
    return unshard(res.results)


# revision 13
# speedup vs baseline: 1.0212x; 1.0212x over previous
"""Causal self-attention (single head) Trainium2 Bass kernel.

Problem: x[4,2048,2048] -> qkv = x@W+b; q,k,v = split(qkv); causal
softmax attention with denominator output.
Returns (y[4,2048,2048], denom[4,2048]) like the reference.

Sharding: 8 cores = batch(4) x query-split(2). Within a batch pair,
each core takes the zebra half of the 16 query chunks (128 rows each):
r=0 -> chunks {g: g%4 in {0,3}}, r=1 -> {g: g%4 in {1,2}}. Both halves
have identical causal key-tile schedules [1,1,2,2,3,3,4,4] (512-wide
key tiles), so a single SPMD program serves all cores with no padding.

K/V projection is split across the core pair (each core projects its
own 1024 key rows) and exchanged with a 2-member AllGather.

All matmuls run in float32r (TF32-like, full bf16 throughput at free
dim >= 256, ~1e-4 relative error).
"""

import sys

for _p in ("/opt/trn_rl_repo", "/root/.axon_site/_ro/trn_rl_repo"):
    if _p not in sys.path:
        sys.path.append(_p)

import numpy as np

import concourse.bass as bass
import concourse.tile as tile
from concourse import mybir
from concourse.bass_utils import run_bass_kernel_spmd

F32 = mybir.dt.float32
F32R = mybir.dt.float32r
BF16 = mybir.dt.bfloat16

B, N, E, D = 4, 2048, 2048, 2048
NCORES = 8
NQ = N // 2              # query rows per core
NKV = N // 2             # key/value rows projected per core
NCHUNK = 16              # 128-row chunks per batch
LCHUNK = 8               # local chunks per core
KTILE = 512
EC = E // 128            # contraction chunks
SCALE = 1.0 / float(np.sqrt(D))
NEG = -1.0e30
GROUPS = [[0, 1], [2, 3], [4, 5], [6, 7]]   # batch pairs

# zebra chunk assignment (identical schedule on every core)
CHUNKS = {
    0: [g for g in range(NCHUNK) if g % 4 in (0, 3)],
    1: [g for g in range(NCHUNK) if g % 4 in (1, 2)],
}
KCOUNTS = [g // 4 + 1 for g in CHUNKS[0]]
assert KCOUNTS == [g // 4 + 1 for g in CHUNKS[1]] == [1, 1, 2, 2, 3, 3, 4, 4]


def _split_waits(nc, limit=1):
    """This walrus build allows only one sem wait per instruction; move
    excess waits onto preceding NoOps on the same engine."""
    for bb in nc.main_func.blocks:
        newl = []
        for ins in bb.instructions:
            si = getattr(ins, "sync_info", None)
            if si is not None and len(si.on_wait) > limit:
                waits = list(si.on_wait)
                for i, w in enumerate(waits[:-limit]):
                    nd = mybir.InstNoOp(name=f"{ins.name}-ws{i}", ins=[], outs=[])
                    nd.engine = ins.engine
                    nd.sync_info = mybir.SyncInfo(on_update=[], on_wait=[w])
                    newl.append(nd)
                si.on_wait = waits[-limit:]
            newl.append(ins)
        bb.instructions = newl
    return nc


def _dma_sliced(nc, tile_ap, dram_ap, nsl=4):
    """DMA a [128, EC, X] tile in nsl ec-group slices so consumers of the
    first chunks can start before the whole transfer lands."""
    step = EC // nsl
    for s in range(nsl):
        nc.sync.dma_start(
            tile_ap[:, s * step : (s + 1) * step, :],
            dram_ap[:, s * step : (s + 1) * step, :],
        )


def _proj_T(nc, P, out_d, x_sb, W, wcol0, bqk_sb, bcol0, nrows):
    """Transposed projection: out_d[cc*128:, rt*512:] = (x @ W_cols).T slices.
    x_sb: [128, EC, nrows] resident input (x^T chunks)."""
    for cc in range(16):
        w_sb = P["wtile"].tile([128, EC, 128], F32R, tag="w", name=f"w_{wcol0}_{cc}")
        nc.sync.dma_start(
            w_sb[:],
            W.ap()[:, wcol0 + cc * 128 : wcol0 + (cc + 1) * 128].rearrange(
                "(a p) c -> p a c", p=128
            ),
        )
        for rt in range(nrows // KTILE):
            ps = P["psA"].tile([128, KTILE], F32, tag="psA", name=f"psA_{wcol0}_{cc}_{rt}")
            for ec in range(EC):
                nc.tensor.matmul(
                    ps[:], w_sb[:, ec, :],
                    x_sb[:, ec, rt * KTILE : (rt + 1) * KTILE],
                    start=(ec == 0), stop=(ec == EC - 1),
                )
            o = P["projo"].tile([128, KTILE], BF16, tag="o", name=f"o_{wcol0}_{cc}_{rt}")
            nc.vector.tensor_scalar_add(o[:], ps[:], bqk_sb[:, bcol0 + cc : bcol0 + cc + 1])
            nc.sync.dma_start(
                out_d[cc * 128 : (cc + 1) * 128, rt * KTILE : (rt + 1) * KTILE], o[:]
            )


def _proj_V(nc, P, out_d, x_sb, W, bv_sb, nrows):
    """Natural-layout projection: out_d[rc*128:, vt*512:] = (x @ Wv) slices."""
    for vt in range(D // KTILE):
        wv_sb = P["wv"].tile([128, EC, KTILE], F32R, tag="wv", name=f"wv_{vt}")
        _dma_sliced(
            nc, wv_sb,
            W.ap()[:, 2 * D + vt * KTILE : 2 * D + (vt + 1) * KTILE].rearrange(
                "(a p) c -> p a c", p=128
            ),
        )
        for rc in range(nrows // 128):
            ps = P["psA"].tile([128, KTILE], F32, tag="psA", name=f"psV_{vt}_{rc}")
            for ec in range(EC):
                nc.tensor.matmul(
                    ps[:], x_sb[:, ec, rc * 128 : (rc + 1) * 128],
                    wv_sb[:, ec, :],
                    start=(ec == 0), stop=(ec == EC - 1),
                )
            o = P["projo"].tile([128, KTILE], BF16, tag="o", name=f"oV_{vt}_{rc}")
            nc.vector.tensor_add(o[:], ps[:], bv_sb[:, vt * KTILE : (vt + 1) * KTILE])
            nc.sync.dma_start(
                out_d[rc * 128 : (rc + 1) * 128, vt * KTILE : (vt + 1) * KTILE], o[:]
            )


def _proj_K_tiled(nc, P, KTh_t, KTg_t, x_sb, W, bqk_sb):
    """K^T projection, rt-outer so each 512-key tile completes early and is
    AllGathered immediately (pipelines comms behind remaining compute).
    KTh_t/KTg_t: per-rt DRAM tiles [D, KTILE] / [2*D, KTILE]."""
    for rt in range(NKV // KTILE):
        for cc in range(16):
            w_sb = P["wtile"].tile([128, EC, 128], F32R, tag="w", name=f"wk_{rt}_{cc}")
            nc.sync.dma_start(
                w_sb[:],
                W.ap()[:, D + cc * 128 : D + (cc + 1) * 128].rearrange(
                    "(a p) c -> p a c", p=128
                ),
            )
            ps = P["psA"].tile([128, KTILE], F32, tag="psA", name=f"psK_{rt}_{cc}")
            for ec in range(EC):
                nc.tensor.matmul(
                    ps[:], w_sb[:, ec, :],
                    x_sb[:, ec, rt * KTILE : (rt + 1) * KTILE],
                    start=(ec == 0), stop=(ec == EC - 1),
                )
            o = P["projo"].tile([128, KTILE], BF16, tag="o", name=f"oK_{rt}_{cc}")
            nc.vector.tensor_scalar_add(o[:], ps[:], bqk_sb[:, 16 + cc : 17 + cc])
            nc.sync.dma_start(KTh_t[rt][cc * 128 : (cc + 1) * 128, :], o[:])
        nc.gpsimd.collective_compute(
            "AllGather", mybir.AluOpType.bypass,
            replica_groups=GROUPS,
            ins=[KTh_t[rt].opt()], outs=[KTg_t[rt].opt()],
        )


def _proj_V_tiled(nc, P, Vh_t, Vg_t, x_sb, W, bv_sb):
    """V projection per 512-column tile; each column tile is AllGathered as
    soon as it is complete. Vh_t/Vg_t: per-vt DRAM tiles [NKV,KTILE]/[N,KTILE]."""
    for vt in range(D // KTILE):
        wv_sb = P["wv"].tile([128, EC, KTILE], F32R, tag="wv", name=f"wv_{vt}")
        _dma_sliced(
            nc, wv_sb,
            W.ap()[:, 2 * D + vt * KTILE : 2 * D + (vt + 1) * KTILE].rearrange(
                "(a p) c -> p a c", p=128
            ),
        )
        for rc in range(NKV // 128):
            ps = P["psA"].tile([128, KTILE], F32, tag="psA", name=f"psV_{vt}_{rc}")
            for ec in range(EC):
                nc.tensor.matmul(
                    ps[:], x_sb[:, ec, rc * 128 : (rc + 1) * 128],
                    wv_sb[:, ec, :],
                    start=(ec == 0), stop=(ec == EC - 1),
                )
            o = P["projo"].tile([128, KTILE], BF16, tag="o", name=f"oV_{vt}_{rc}")
            nc.vector.tensor_add(o[:], ps[:], bv_sb[:, vt * KTILE : (vt + 1) * KTILE])
            nc.sync.dma_start(Vh_t[vt][rc * 128 : (rc + 1) * 128, :], o[:])
        nc.gpsimd.collective_compute(
            "AllGather", mybir.AluOpType.bypass,
            replica_groups=GROUPS,
            ins=[Vh_t[vt].opt()], outs=[Vg_t[vt].opt()],
        )


def _attn_pass(nc, P, j, lc, kt, kcount, qt_sb, kt_sb, v_sb, mask_sb,
               m_run, d_run, y_acc, ident_sb):
    """One [128q x 512k] attention pass for local chunk lc (index j within
    its group), key tile kt."""
    diag = kt == kcount - 1
    ps_s = P["psS"].tile([128, KTILE], F32, tag="psS", name=f"psS_{lc}_{kt}")
    for ec in range(EC):
        nc.tensor.matmul(
            ps_s[:], qt_sb[:, ec, j * 128 : (j + 1) * 128], kt_sb[:, ec, :],
            start=(ec == 0), stop=(ec == EC - 1),
        )
    if diag:
        src = P["soft"].tile([128, KTILE], F32, tag="masked", name=f"mskd_{lc}_{kt}")
        nc.vector.tensor_add(src[:], ps_s[:], mask_sb[j][:])
    else:
        src = ps_s
    # running max of raw logits (for the reference-matching denominator)
    mx = P["stats"].tile([128, 1], F32, tag="mx", name=f"mx_{lc}_{kt}")
    nc.vector.tensor_reduce(
        mx[:], src[:], axis=mybir.AxisListType.X, op=mybir.AluOpType.max
    )
    if kt == 0:
        nc.vector.tensor_copy(m_run[j][:], mx[:])
    else:
        nc.vector.tensor_max(m_run[j][:], m_run[j][:], mx[:])
    # exp without max subtraction (logits are O(5), fp32-safe) + denom part
    p_sb = P["soft"].tile([128, KTILE], BF16, tag="p", name=f"p_{lc}_{kt}")
    dpart = P["stats"].tile([128, 1], F32, tag="dpart", name=f"dp_{lc}_{kt}")
    nc.scalar.activation(
        p_sb[:], src[:], mybir.ActivationFunctionType.Exp,
        scale=SCALE, accum_out=dpart[:],
    )
    if kt == 0:
        nc.vector.tensor_copy(d_run[j][:], dpart[:])
    else:
        nc.vector.tensor_add(d_run[j][:], d_run[j][:], dpart[:])
    # transpose P -> PT (4 x 128x128, PE transpose via identity)
    pt_sb = P["soft"].tile([128, 4, 128], BF16, tag="pt", name=f"pt_{lc}_{kt}")
    for c in range(4):
        ps_t = P["psT"].tile([128, 128], BF16, tag="psT", name=f"psT_{lc}_{kt}_{c}")
        nc.tensor.transpose(ps_t[:], p_sb[:, c * 128 : (c + 1) * 128], ident_sb[:])
        nc.vector.tensor_copy(pt_sb[:, c, :], ps_t[:])
    # PV for this key tile, accumulate into y_acc
    for dt in range(4):
        ps_y = P["psY"].tile([128, KTILE], F32, tag="psY", name=f"psY_{lc}_{kt}_{dt}")
        for c in range(4):
            nc.tensor.matmul(
                ps_y[:], pt_sb[:, c, :], v_sb[:, c, dt * KTILE : (dt + 1) * KTILE],
                start=(c == 0), stop=(c == 3),
            )
        dsl = slice(dt * KTILE, (dt + 1) * KTILE)
        if kt == 0:
            nc.vector.tensor_copy(y_acc[j][:, dsl], ps_y[:])
        else:
            nc.vector.tensor_add(y_acc[j][:, dsl], y_acc[j][:, dsl], ps_y[:])


def _phase_b(nc, P, QT_d, KTg_t, Vg_t, msk, ident_sb, y_out, den_out):
    """Attention over 2 groups of 4 local chunks; key tiles streamed once
    per group. KTg_t[s] is gathered K^T for key sub-tile s of each half
    ([2*D, KTILE], half-major); Vg_t[vt] is gathered V columns vt
    ([N, KTILE], natural rows)."""
    for grp in range(2):
        lcs = [4 * grp + j for j in range(4)]          # local chunks
        kcs = [KCOUNTS[lc] for lc in lcs]
        max_kt = kcs[-1]                                # tiles this group needs
        qt_sb = P["qt"].tile([128, EC, 512], BF16, tag="qt", name=f"qt_{grp}")
        _dma_sliced(
            nc, qt_sb,
            QT_d[:, grp * 512 : (grp + 1) * 512].rearrange("(a p_) r -> p_ a r", p_=128),
        )
        y_acc = [P["yacc"].tile([128, D], F32, tag=f"yacc{j}", name=f"ya_{grp}_{j}")
                 for j in range(4)]
        m_run = [P["stats"].tile([128, 1], F32, tag=f"mrun{j}", name=f"mr_{grp}_{j}")
                 for j in range(4)]
        d_run = [P["stats"].tile([128, 1], F32, tag=f"drun{j}", name=f"dr_{grp}_{j}")
                 for j in range(4)]
        mask_sb = [P["mskp"].tile([128, KTILE], F32, tag=f"msk{j}", name=f"mk_{grp}_{j}")
                   for j in range(4)]
        for j in range(4):
            nc.sync.dma_start(mask_sb[j][:], msk.ap()[4 * grp + j])

        for kt in range(max_kt):
            # key tile kt lives in gathered half kt//2, sub-block kt%2
            h, s = divmod(kt, 2)
            kt_sb = P["kt"].tile([128, EC, KTILE], BF16, tag="ktile",
                                 name=f"kt_{grp}_{kt}")
            _dma_sliced(
                nc, kt_sb,
                KTg_t[s][h * D : (h + 1) * D, :].rearrange(
                    "(a p_) r -> p_ a r", p_=128
                ),
            )
            v_sb = P["vt"].tile([128, 4, D], BF16, tag="vtile", name=f"v_{grp}_{kt}")
            for c in range(4):
                for vt in range(4):
                    nc.sync.dma_start(
                        v_sb[:, c, vt * KTILE : (vt + 1) * KTILE],
                        Vg_t[vt][kt * KTILE + c * 128 : kt * KTILE + (c + 1) * 128, :],
                    )
            for j, lc in enumerate(lcs):
                if kt >= kcs[j]:
                    continue
                _attn_pass(nc, P, j, lc, kt, kcs[j], qt_sb, kt_sb, v_sb, mask_sb,
                           m_run, d_run, y_acc, ident_sb)

        # group epilogue: normalize + outputs
        for j, lc in enumerate(lcs):
            rden = P["stats"].tile([128, 1], F32, tag="rden", name=f"rd_{grp}_{j}")
            nc.vector.reciprocal(rden[:], d_run[j][:])
            yo = P["yo"].tile([128, D], F32, tag="yo", name=f"yo_{grp}_{j}")
            nc.vector.tensor_scalar_mul(yo[:], y_acc[j][:], rden[:])
            nc.sync.dma_start(y_out.ap()[lc * 128 : (lc + 1) * 128, :], yo[:])
            em = P["stats"].tile([128, 1], F32, tag="em", name=f"em_{grp}_{j}")
            nc.scalar.activation(
                em[:], m_run[j][:], mybir.ActivationFunctionType.Exp, scale=-SCALE
            )
            dfin = P["stats"].tile([128, 1], F32, tag="dfin", name=f"df_{grp}_{j}")
            nc.vector.tensor_mul(dfin[:], d_run[j][:], em[:])
            nc.sync.dma_start(den_out.ap()[lc * 128 : (lc + 1) * 128, :], dfin[:])


def build_nc(repeat=1):
    """Build the SPMD program. repeat>1 repeats the whole body (for slope
    timing of device time, immune to dispatch overhead)."""
    nc = bass.Bass("TRN2", target_bir_lowering=False, debug=False, num_devices=8)

    xkvT = nc.dram_tensor("xkvT", [E, NKV], F32R, kind="ExternalInput")
    xqT = nc.dram_tensor("xqT", [E, NQ], F32R, kind="ExternalInput")
    W = nc.dram_tensor("W", [E, 3 * D], F32R, kind="ExternalInput")
    bqk = nc.dram_tensor("bqk", [128, 32], F32, kind="ExternalInput")
    bv = nc.dram_tensor("bv", [128, D], F32, kind="ExternalInput")
    msk = nc.dram_tensor("msk", [LCHUNK, 128, KTILE], F32, kind="ExternalInput")
    ident = nc.dram_tensor("ident", [128, 128], BF16, kind="ExternalInput")

    y_out = nc.dram_tensor("y_out", [NQ, D], F32, kind="ExternalOutput")
    den_out = nc.dram_tensor("den_out", [NQ, 1], F32, kind="ExternalOutput")

    with tile.TileContext(nc) as tc:
        with (
            tc.tile_pool(name="dram", bufs=1, space="DRAM") as dram,
            tc.tile_pool(name="const", bufs=1) as const_p,
        ):
            QT_d = dram.tile([D, NQ], BF16)
            KTh_t = [dram.tile([D, KTILE], BF16, tag=f"kth{s}", name=f"kth{s}")
                     for s in range(NKV // KTILE)]
            KTg_t = [dram.tile([2 * D, KTILE], BF16, tag=f"ktg{s}", name=f"ktg{s}")
                     for s in range(NKV // KTILE)]
            Vh_t = [dram.tile([NKV, KTILE], BF16, tag=f"vh{v}", name=f"vh{v}")
                    for v in range(D // KTILE)]
            Vg_t = [dram.tile([N, KTILE], BF16, tag=f"vg{v}", name=f"vg{v}")
                    for v in range(D // KTILE)]

            ident_sb = const_p.tile([128, 128], BF16)
            nc.sync.dma_start(ident_sb[:], ident.ap())

            for it in range(repeat):
                with (
                    tc.tile_pool(name=f"xkv{it}", bufs=1) as xkv_p,
                    tc.tile_pool(name=f"xq{it}", bufs=1) as xq_p,
                    tc.tile_pool(name=f"wtile{it}", bufs=3) as wtile_p,
                    tc.tile_pool(name=f"wv{it}", bufs=1) as wv_p,
                    tc.tile_pool(name=f"projo{it}", bufs=3) as projo_p,
                    tc.tile_pool(name=f"constA{it}", bufs=1) as constA_p,
                    tc.tile_pool(name=f"psA{it}", bufs=4, space="PSUM") as psA,
                ):
                    P = {"wtile": wtile_p, "wv": wv_p, "projo": projo_p, "psA": psA}
                    bqk_sb = constA_p.tile([128, 32], F32)
                    nc.sync.dma_start(bqk_sb[:], bqk.ap())
                    bv_sb = constA_p.tile([128, D], F32)
                    nc.sync.dma_start(bv_sb[:], bv.ap())

                    xkv_sb = xkv_p.tile([128, EC, NKV], F32R)
                    _dma_sliced(nc, xkv_sb, xkvT.ap().rearrange("(a p) r -> p a r", p=128))

                    # K and V tiles gather as they complete; Q proj overlaps comms
                    _proj_K_tiled(nc, P, KTh_t, KTg_t, xkv_sb, W, bqk_sb)
                    # xq load here: overlaps V projection, off K's W-tile queues
                    xq_sb = xq_p.tile([128, EC, NQ], F32R)
                    _dma_sliced(nc, xq_sb, xqT.ap().rearrange("(a p) r -> p a r", p=128))
                    _proj_V_tiled(nc, P, Vh_t, Vg_t, xkv_sb, W, bv_sb)
                    _proj_T(nc, P, QT_d, xq_sb, W, 0, bqk_sb, 0, NQ)

                with (
                    tc.tile_pool(name=f"kt{it}", bufs=2) as kt_p,
                    tc.tile_pool(name=f"vt{it}", bufs=2) as vt_p,
                    tc.tile_pool(name=f"qt{it}", bufs=1) as qt_p,
                    tc.tile_pool(name=f"yacc{it}", bufs=1) as yacc_p,
                    tc.tile_pool(name=f"soft{it}", bufs=3) as soft_p,
                    tc.tile_pool(name=f"yo{it}", bufs=1) as yo_p,
                    tc.tile_pool(name=f"stats{it}", bufs=16) as stats_p,
                    tc.tile_pool(name=f"mskp{it}", bufs=1) as msk_p,
                    tc.tile_pool(name=f"psS{it}", bufs=2, space="PSUM") as psS,
                    tc.tile_pool(name=f"psT{it}", bufs=2, space="PSUM") as psT,
                    tc.tile_pool(name=f"psY{it}", bufs=2, space="PSUM") as psY,
                ):
                    P = {"kt": kt_p, "vt": vt_p, "qt": qt_p, "yacc": yacc_p,
                         "soft": soft_p, "yo": yo_p, "stats": stats_p,
                         "mskp": msk_p, "psS": psS, "psT": psT, "psY": psY}
                    _phase_b(nc, P, QT_d, KTg_t, Vg_t, msk, ident_sb, y_out, den_out)

    _split_waits(nc)
    return nc


_NC_CACHE = None


def _get_nc():
    global _NC_CACHE
    if _NC_CACHE is None:
        _NC_CACHE = build_nc()
    return _NC_CACHE


def make_in_maps(x, W, b):
    x = np.asarray(x, dtype=np.float32)
    W = np.asarray(W, dtype=np.float32)
    b = np.asarray(b, dtype=np.float32)
    bqk = np.ascontiguousarray(b[: 2 * D].reshape(32, 128).T)
    bv = np.broadcast_to(b[2 * D :], (128, D)).copy()
    import ml_dtypes
    ident = np.eye(128, dtype=ml_dtypes.bfloat16)
    in_maps = []
    for core in range(NCORES):
        bb, r = divmod(core, 2)
        chunks = CHUNKS[r]
        qrows = np.concatenate([np.arange(g * 128, (g + 1) * 128) for g in chunks])
        xkvT = np.ascontiguousarray(x[bb][r * NKV : (r + 1) * NKV].T)
        xqT = np.ascontiguousarray(x[bb][qrows].T)
        msk = np.empty((LCHUNK, 128, KTILE), dtype=np.float32)
        for lc, g in enumerate(chunks):
            kc = KCOUNTS[lc]
            gi = g * 128 + np.arange(128)[:, None]
            kj = (kc - 1) * KTILE + np.arange(KTILE)[None, :]
            msk[lc] = np.where(kj <= gi, 0.0, NEG)
        in_maps.append(
            {"xkvT": xkvT, "xqT": xqT, "W": W, "bqk": bqk, "bv": bv,
             "msk": msk, "ident": ident}
        )
    return in_maps


def unshard(results):
    y = np.empty((B, N, D), dtype=np.float32)
    denom = np.empty((B, N), dtype=np.float32)
    for core in range(NCORES):
        bb, r = divmod(core, 2)
        chunks = CHUNKS[r]
        qrows = np.concatenate([np.arange(g * 128, (g + 1) * 128) for g in chunks])
        y[bb][qrows] = results[core]["y_out"]
        denom[bb][qrows] = results[core]["den_out"][:, 0]
    return y, denom


def kernel(x, W, b):
    nc = _get_nc()
    in_maps = make_in_maps(x, W, b)
    res = run_bass_kernel_spmd(nc, in_maps, list(range(NCORES)))
    return unshard(res.results)
